# revision 2
# baseline (speedup 1.0000x reference)
"""Trainium2 Bass kernel for nn_GATv2Base (gnn_message_passing) — v2.

Contract: kernel(**inputs) takes FULL unsharded inputs (same keys as
reference.setup_inputs()) and returns the FULL [32, 64] float32 output.

Sharding: 32 graphs -> 8 cores (4 graphs each, contiguous node ranges since
`batch` is sorted).  Edges (plus self-loops) are routed to the core owning
their dst node, sorted by dst, and packed into spans (<=127-node dst window,
2304 edge slots = 18 subgroups of 128; subgroups 0-8 hold edges whose src row
is in the lower half of the padded node table, 9-17 the upper half).

Per span, the source-feature gather xl[src] runs on BOTH descriptor-generation
engines in parallel: the first 7 subgroups of each half via SWDGE dma_gather
(4 queues, int16 half-relative indices), the last 2 via hardware-DGE
indirect_dma_start (int32 global indices).  The dst-side term xr[dst] is NOT
gathered: each dst row lives in the span's own 128-row window, so it is
expanded edge-wise on the PE with one-hot matrices built from the dst column
(S for the scatter direction, S^T via PE transpose for the broadcast
direction).

Layer 1 output h1 never goes to DRAM: the span sink transposes it on the PE
and immediately computes xl2/xr2.  One fp16 AllGather shares the xl2 table;
the pooled per-graph MLP runs on the owning core.  Only [4, 64] per-core
outputs return to the host.
"""

import os
import sys

import numpy as np

for _p in ("/opt/trn_rl_repo", "/root/.axon_site/_ro/trn_rl_repo"):
    if os.path.isdir(_p) and _p not in sys.path:
        sys.path.insert(0, _p)

import concourse.bass as bass
import concourse.bacc as bacc
import concourse.mybir as mybir
import concourse.tile as tile
from concourse.bass import IndirectOffsetOnAxis
from concourse.bass_utils import run_bass_kernel_spmd
from concourse.masks import make_identity

F32 = mybir.dt.float32
F16 = mybir.dt.float16
I32 = mybir.dt.int32
I16 = mybir.dt.int16
AF = mybir.ActivationFunctionType
ALU = mybir.AluOpType
AXX = mybir.AxisListType.X

N, E, H, C, NG = 50000, 800000, 4, 64, 32
HC = H * C
NCORES = 8
SLOT_H = 1152            # edge slots per table-half region (9 subgroups)
SPAN_EDGES = 2 * SLOT_H  # 2304 edge slots per span
NSG = SPAN_EDGES // 128  # 18
HSG = NSG // 2           # 9
NQP = 2                  # subgroups per half gathered via qPoolDynamic
NSW = HSG - NQP          # subgroups per half gathered via SWDGE
SPAN_DST = 127           # dst window per span; 127 = pad marker
_PAD0 = True             # pad swdge idx with 0 (-1 auto-trim crashes this ucode)
_REGTRIM = False         # reg/-1 desc trimming both crash this ucode build


# ----------------------------------------------------------------------------
# Host-side sharding / packing
# ----------------------------------------------------------------------------

def _host_prep(inp):
    x = np.asarray(inp["x"], dtype=np.float32)
    ei = np.asarray(inp["edge_index"], dtype=np.int32)
    ea_full = np.asarray(inp["edge_attr"], dtype=np.float32)[:, 0]
    batch = np.asarray(inp["batch"], dtype=np.int32)

    src0, dst0 = ei[0], ei[1]
    deg = np.maximum(np.bincount(dst0, minlength=N).astype(np.float64), 1.0)
    loop_attr = (
        np.bincount(dst0, weights=ea_full.astype(np.float64), minlength=N) / deg
    ).astype(np.float32)
    src = np.concatenate([src0, np.arange(N, dtype=np.int32)])
    dst = np.concatenate([dst0, np.arange(N, dtype=np.int32)])
    eattr = np.concatenate([ea_full, loop_attr]).astype(np.float32)

    gcounts = np.bincount(batch, minlength=NG)
    gstart = np.concatenate([[0], np.cumsum(gcounts)])
    core_n0 = np.array([gstart[4 * k] for k in range(NCORES)] + [N], dtype=np.int64)

    order = np.argsort(dst, kind="stable")
    src, dst, eattr = src[order], dst[order], eattr[order]
    edge_lo = np.searchsorted(dst, core_n0[:-1], "left")
    edge_hi = np.searchsorted(dst, core_n0[1:], "left")

    src_owner = np.searchsorted(core_n0[1:], src, "right")
    src_in_a = src_owner < (NCORES // 2)

    cores = []
    for k in range(NCORES):
        n0, n1 = int(core_n0[k]), int(core_n0[k + 1])
        s, e = int(edge_lo[k]), int(edge_hi[k])
        cd = dst[s:e]
        ca = src_in_a[s:e]
        nlocal = n1 - n0
        node_edge_start = np.searchsorted(cd, n0 + np.arange(nlocal + 1))
        cumA = np.concatenate([[0], np.cumsum(ca)])
        spans = []
        b = 0
        while b < nlocal:
            bend = b
            while bend < nlocal and (bend - b) < SPAN_DST:
                e0, e1 = node_edge_start[b], node_edge_start[bend + 1]
                nA = cumA[e1] - cumA[e0]
                nB = (e1 - e0) - nA
                if nA > SLOT_H or nB > SLOT_H:
                    break
                bend += 1
            assert bend > b, "single node exceeds span edge capacity"
            spans.append(
                (b, bend - b, int(node_edge_start[b]), int(node_edge_start[bend]))
            )
            b = bend
        cores.append(
            dict(n0=n0, n1=n1, spans=spans, src=src[s:e], dst=cd, ea=eattr[s:e],
                 in_a=ca)
        )

    nspans = max(len(c["spans"]) for c in cores)
    rows_per_core = nspans * 128
    rows_total = NCORES * rows_per_core
    half_rows = rows_total // 2
    assert half_rows <= 32767, f"table half {half_rows} exceeds int16 index range"

    node_row = np.zeros(N, dtype=np.int64)
    for k, c in enumerate(cores):
        for si, (b, nb, _, _) in enumerate(c["spans"]):
            nodes = np.arange(c["n0"] + b, c["n0"] + b + nb)
            node_row[nodes] = k * rows_per_core + si * 128 + (nodes - c["n0"] - b)

    x_pad = np.zeros((rows_total, 4), dtype=np.float32)
    x_pad[node_row] = x
    x_aug_T = np.concatenate(
        [x_pad.T, np.ones((1, rows_total), dtype=np.float32)], axis=0
    )  # [5, R]

    def wrap16(vals):
        # [n] -> wrapped [128, n//16] int16 (16-part wrap, replicated x8)
        n = len(vals)
        base = np.zeros((16, n // 16), dtype=np.int16)
        i = np.arange(n)
        base[i % 16, i // 16] = vals.astype(np.int16)
        return np.tile(base, (8, 1))

    sw_cols = NSW * 128 // 16  # idx cols per half for the SWDGE portion
    packs = []
    for k, c in enumerate(cores):
        idxsw = np.zeros((nspans, 128, 2, sw_cols), np.int16)
        idxqp = np.zeros((nspans, 128, 2 * NQP), np.int32)
        swcnt = np.full((nspans, 1, 4), 16, np.int32)
        metaF = np.zeros((nspans, 128, 40), np.float16)
        metaF[:, :, 0:NSG] = 127.0  # dcol pad marker
        for si, (b, nb, e0, e1) in enumerate(c["spans"]):
            ina = c["in_a"][e0:e1]
            esrc = node_row[c["src"][e0:e1]]
            edrel = (c["dst"][e0:e1] - c["n0"] - b).astype(np.int64)
            eea = c["ea"][e0:e1]
            # slots: A edges at [0, SLOT_H), B edges at [SLOT_H, 2*SLOT_H)
            ia = np.where(ina)[0]
            ib = np.where(~ina)[0]
            slots = np.empty(len(ina), dtype=np.int64)
            slots[ia] = np.arange(len(ia))
            slots[ib] = SLOT_H + np.arange(len(ib))
            for h, iv in ((0, ia), (1, ib)):
                # qpool takes the FIRST NQP subgroups (always full of real
                # edges), swdge the rest: trailing pads are marked -1 so the
                # q7 desc-gen kernel auto-trims them.
                rows_h = np.full(SLOT_H, -1, dtype=np.int64)
                rows_h[:len(iv)] = esrc[iv] - h * half_rows
                qp = rows_h[:NQP * 128] + h * half_rows  # global rows
                qp[qp < 0] = 0
                idxqp[si, :, h * NQP:(h + 1) * NQP] = qp.reshape(NQP, 128).T
                sw = rows_h[NQP * 128:]
                sw[sw < 0] = 0 if _PAD0 else -1
                idxsw[si, :, h, :] = wrap16(sw)
                hs_ = (HSG - NQP) // 2
                nH = len(iv)
                swcnt[si, 0, 2 * h + 0] = np.clip(nH - NQP * 128, 16, hs_ * 128)
                swcnt[si, 0, 2 * h + 1] = np.clip(
                    nH - (NQP + hs_) * 128, 16, (HSG - NQP - hs_) * 128)
            p, sg = slots % 128, slots // 128
            metaF[si, p, sg] = edrel.astype(np.float16)            # dcol
            metaF[si, p, NSG + sg] = eea.astype(np.float16)        # eac
            nodes = np.arange(c["n0"] + b, c["n0"] + b + nb)
            gl = batch[nodes] - 4 * k
            metaF[si, np.arange(nb), 2 * NSG + gl] = np.float16(1.0)  # gmask
        inv_cnt = np.zeros((4, 1), dtype=np.float32)
        for gg in range(4):
            cnt = max(int(gcounts[4 * k + gg]), 1)
            inv_cnt[gg, 0] = 1.0 / cnt
        packs.append(
            dict(
                idxsw=idxsw,
                idxqp=idxqp,
                swcnt=swcnt,
                metaF=metaF,
                inv_cnt=inv_cnt,
                own_cols=np.arange(
                    k * rows_per_core, (k + 1) * rows_per_core, dtype=np.int64
                ),
            )
        )
    return cores, packs, nspans, rows_per_core, rows_total, x_aug_T, node_row


# ----------------------------------------------------------------------------
# Device program
# ----------------------------------------------------------------------------

_PROGRAM_CACHE = {}


def _build_program(nspans, rows_total, repeat=1, phase_limit=9):
    rows_per_core = nspans * 128
    nblocks = rows_total // 128
    half_rows = rows_total // 2
    sw_cols = NSW * 128 // 16

    nc = bacc.Bacc(num_swdge_queues=4)
    tcx = tile.TileContext(nc)

    def din(name, shape, dt):
        return nc.dram_tensor(name, shape, dt, kind="ExternalInput")

    t_xaugT = din("xaugT", [5, rows_total], F32)
    t_own_xaugT = din("own_xaugT", [5, rows_per_core], F32)
    t_enc_aug = din("enc_aug", [5, 64], F32)
    t_w1 = {}
    t_w2 = {}
    for L, tw in ((1, t_w1), (2, t_w2)):
        kdim = 65 if L == 1 else 257
        tw["wl_aug"] = din(f"wl{L}_aug", [kdim, HC], F16)
        tw["wr_aug"] = din(f"wr{L}_aug", [kdim, HC], F16)
        tw["att_row"] = din(f"att{L}_row", [128, HC], F16)
        tw["we_row"] = din(f"we{L}_row", [128, HC], F16)
        tw["bias_row"] = din(f"bias{L}_row", [128, HC], F16)
    t_idxsw = din("idxsw", [nspans, 128, 2, sw_cols], I16)
    t_idxqp = din("idxqp", [nspans, 128, 2 * NQP], I32)
    t_swcnt = din("swcnt", [nspans, 1, 4], I32)
    t_metaF = din("metaF", [nspans, 128, 40], F16)
    t_iota_row = din("iota_row", [128, 128], F16)
    t_inv_cnt = din("inv_cnt", [4, 1], F32)
    t_p1_aug = din("p1_aug", [257, 128], F32)
    t_ln_g = din("ln_g4", [4, 128], F32)
    t_ln_b = din("ln_b4", [4, 128], F32)
    t_p2_aug = din("p2_aug", [129, 64], F32)
    t_out = nc.dram_tensor("out", [4, 64], F32, kind="ExternalOutput")

    # internal DRAM
    t_xl1 = nc.dram_tensor("xl1_tbl", [rows_total, HC], F16)
    t_xr1 = nc.dram_tensor("xr1_own", [rows_per_core, HC], F16)
    t_xr2 = nc.dram_tensor("xr2_own", [rows_per_core, HC], F16)
    t_xl2_in = nc.dram_tensor("xl2_own_cc", [rows_per_core, HC], F16)
    t_xl2 = nc.dram_tensor("xl2_tbl", [rows_total, HC], F16, addr_space="Shared")

    from contextlib import ExitStack
    with tcx as tc, ExitStack() as es:
        # constants
        cpool = es.enter_context(tc.tile_pool(name="consts", bufs=1))
        enc_aug = cpool.tile([5, 64], F32)
        nc.sync.dma_start(out=enc_aug[:], in_=t_enc_aug[:])
        iota_rep = cpool.tile([128, 128], F16)
        nc.sync.dma_start(out=iota_rep[:], in_=t_iota_row[:])
        reps = {}
        for L, tw in ((1, t_w1), (2, t_w2)):
            for nm in ("att_row", "we_row", "bias_row"):
                rep = cpool.tile([128, HC], F16, tag=f"rep{L}{nm}")
                nc.sync.dma_start(out=rep[:], in_=tw[nm][:])
                reps[(L, nm)] = rep
        ones_col = cpool.tile([1, 128], F16)
        nc.vector.memset(ones_col[:], 1.0)
        identF = cpool.tile([128, 128], F16)
        make_identity(nc, identF[:])

        wpool = es.enter_context(tc.tile_pool(name="weights", bufs=1))
        wl1 = wpool.tile([65, HC], F16)
        wr1 = wpool.tile([65, HC], F16)
        nc.sync.dma_start(out=wl1[:], in_=t_w1["wl_aug"][:])
        nc.sync.dma_start(out=wr1[:], in_=t_w1["wr_aug"][:])
        w2_tiles = {}
        for nm in ("wl_aug", "wr_aug"):
            a = wpool.tile([128, HC], F16, tag=f"{nm}a")
            b = wpool.tile([128, HC], F16, tag=f"{nm}b")
            cb = wpool.tile([1, HC], F16, tag=f"{nm}c")
            nc.sync.dma_start(out=a[:], in_=t_w2[nm][0:128, :])
            nc.sync.dma_start(out=b[:], in_=t_w2[nm][128:256, :])
            nc.sync.dma_start(out=cb[:], in_=t_w2[nm][256:257, :])
            w2_tiles[nm] = (a, b, cb)

        for _rep in range(repeat):
            _build_iteration(
                nc, tc, nspans, rows_total, rows_per_core, nblocks, half_rows,
                sw_cols, enc_aug, iota_rep, identF, reps, ones_col, wl1, wr1,
                w2_tiles,
                t_xaugT, t_own_xaugT, t_idxsw, t_idxqp, t_swcnt, t_metaF,
                t_inv_cnt,
                t_p1_aug, t_ln_g, t_ln_b, t_p2_aug, t_out,
                t_xl1, t_xr1, t_xr2, t_xl2_in, t_xl2, phase_limit,
            )

    nc.finalize()
    return nc


def _build_iteration(
    nc, tc, nspans, rows_total, rows_per_core, nblocks, half_rows, sw_cols,
    enc_aug, iota_rep, identF, reps, ones_col, wl1, wr1, w2_tiles,
    t_xaugT, t_own_xaugT, t_idxsw, t_idxqp, t_swcnt, t_metaF, t_inv_cnt,
    t_p1_aug, t_ln_g, t_ln_b, t_p2_aug, t_out,
    t_xl1, t_xr1, t_xr2, t_xl2_in, t_xl2, phase_limit=9,
):
    swregs = None
    if _REGTRIM:
        swregs = []
        for j in range(4):
            _swreg = nc.alloc_register(
                mybir.EngineType.Pool, f"swcnt{j}_{nc.next_id()}")
            swregs.append(_swreg)
    # ------------------------------------------------------------------
    # Phase 1: encoder + xl1 for ALL rows (replicated) + own xr1
    # ------------------------------------------------------------------
    def encode_block(pool, ppool, xaugT_ap):
        xT = pool.tile([5, 128], F32, tag="xT")
        nc.sync.dma_start(out=xT[:], in_=xaugT_ap)
        h0psum = ppool.tile([64, 128], F32, tag="h0ps")
        nc.tensor.matmul(out=h0psum[:], lhsT=enc_aug[:], rhs=xT[:],
                         start=True, stop=True)
        h0T = pool.tile([65, 128], F16, tag="h0T")
        nc.scalar.activation(out=h0T[0:64, :], in_=h0psum[:], func=AF.Relu)
        nc.vector.tensor_copy(out=h0T[64:65, :], in_=ones_col[:])
        return h0T

    with tc.tile_pool(name="p1", bufs=3) as pool, \
         tc.tile_pool(name="p1ps", bufs=2, space="PSUM") as ppool:
        for blk in range(nblocks):
            h0T = encode_block(pool, ppool, t_xaugT[:, blk * 128:(blk + 1) * 128])
            xlp = ppool.tile([128, HC], F32, tag="xlps")
            nc.tensor.matmul(out=xlp[:], lhsT=h0T[:], rhs=wl1[:],
                             start=True, stop=True)
            xls = pool.tile([128, HC], F16, tag="xls")
            if blk % 2 == 0:
                nc.vector.tensor_copy(out=xls[:], in_=xlp[:])
            else:
                nc.scalar.copy(out=xls[:], in_=xlp[:])
            nc.sync.dma_start(
                out=t_xl1[blk * 128:(blk + 1) * 128, :], in_=xls[:]
            )
        for s in range(nspans):
            h0T = encode_block(pool, ppool, t_own_xaugT[:, s * 128:(s + 1) * 128])
            xrp = ppool.tile([128, HC], F32, tag="xlps")
            nc.tensor.matmul(out=xrp[:], lhsT=h0T[:], rhs=wr1[:],
                             start=True, stop=True)
            xrs = pool.tile([128, HC], F16, tag="xls")
            nc.vector.tensor_copy(out=xrs[:], in_=xrp[:])
            nc.sync.dma_start(
                out=t_xr1[s * 128:(s + 1) * 128, :], in_=xrs[:]
            )

    # ------------------------------------------------------------------
    # GAT span loop (both layers)
    # ------------------------------------------------------------------
    def gat_layer(L, xl_tbl, xr_tbl, h_sink):
        att_rep = reps[(L, "att_row")]
        we_rep = reps[(L, "we_row")]
        bias_rep = reps[(L, "bias_row")]
        with tc.tile_pool(name=f"g{L}", bufs=2) as pool, \
             tc.tile_pool(name=f"g{L}b", bufs=3) as spool, \
             tc.tile_pool(name=f"g{L}st", bufs=2, space="PSUM") as stpool, \
             tc.tile_pool(name=f"g{L}ps", bufs=2, space="PSUM") as ppool:
            for s in range(nspans):
                iw = spool.tile([128, 2, sw_cols], I16, tag="iw")
                nc.sync.dma_start(out=iw[:], in_=t_idxsw[s])
                iq = spool.tile([128, 2 * NQP], I32, tag="iq")
                nc.sync.dma_start(out=iq[:], in_=t_idxqp[s])
                mf = spool.tile([128, 40], F16, tag="mf")
                nc.sync.dma_start(out=mf[:], in_=t_metaF[s])
                if _REGTRIM:
                    cnt = spool.tile([1, 4], I32, tag="cnt")
                    nc.sync.dma_start(out=cnt[:], in_=t_swcnt[s])
                xr_fl = spool.tile([128, HC], F16, tag="xrfl")
                nc.sync.dma_start(
                    out=xr_fl[:], in_=xr_tbl[s * 128:(s + 1) * 128, :]
                )

                # ---- G = xl[src] gather: qPoolDynamic (first NQP subgroups
                # per half) + SWDGE on 4 queues (rest; trailing pads trimmed)
                G = pool.tile([128, NSG, HC], F16, tag="G")
                if s < 2:
                    # pad slots trimmed from the gather leave stale SBUF
                    # bytes; seed the two ring buffers once so they are
                    # always finite.
                    nc.vector.memset(G[:].rearrange("p a b -> p (a b)"), 0.0)
                hs = NSW // 2  # 3|4 split of the SWDGE subgroups
                for h in range(2):
                    base = h * HSG
                    tbl_half = (xl_tbl[0:half_rows, :] if h == 0
                                else xl_tbl[half_rows:, :])
                    for j in range(NQP):
                        nc.gpsimd.indirect_dma_start(
                            out=G[:, base + j, :],
                            out_offset=None,
                            in_=xl_tbl[:],
                            in_offset=IndirectOffsetOnAxis(
                                ap=iq[:, h * NQP + j:h * NQP + j + 1], axis=0),
                        )
                    if _REGTRIM:
                        nc.gpsimd.reg_load(swregs[2 * h + 0],
                                           cnt[0:1, 2 * h + 0:2 * h + 1])
                        nc.gpsimd.reg_load(swregs[2 * h + 1],
                                           cnt[0:1, 2 * h + 1:2 * h + 2])
                    r0 = swregs[2 * h + 0] if _REGTRIM else hs * 128
                    r1 = swregs[2 * h + 1] if _REGTRIM else (NSW - hs) * 128
                    nc.gpsimd.dma_gather(
                        G[:, base + NQP:base + NQP + hs, :], tbl_half,
                        iw[:, h, 0:hs * 8], hs * 128, r0, HC,
                        single_packet=False, queue_num=2 * h)
                    nc.gpsimd.dma_gather(
                        G[:, base + NQP + hs:base + HSG, :], tbl_half,
                        iw[:, h, hs * 8:NSW * 8], (NSW - hs) * 128,
                        r1, HC,
                        single_packet=False, queue_num=2 * h + 1)

                # ---- S one-hot [e_part, sg, d] (one fused DVE op)
                S = pool.tile([128, NSG, 128], F16, tag="S")
                dco = mf[:, 0:NSG].rearrange(
                    "p (a o) -> p a o", o=1).broadcast_to((128, NSG, 128))
                iot = iota_rep[:].rearrange(
                    "p (o c) -> p o c", o=1).broadcast_to((128, NSG, 128))
                nc.vector.tensor_tensor(out=S[:], in0=dco, in1=iot,
                                        op=ALU.is_equal)

                # ---- R expansion + v = we*ea + R   (per subgroup)
                v = pool.tile([128, NSG, HC], F16, tag="v")
                for sg in range(NSG):
                    stps = stpool.tile([128, 128], F16, tag="stps")
                    nc.tensor.transpose(out=stps[:], in_=S[:, sg, :],
                                        identity=identF[:])
                    st = pool.tile([128, 128], F16, tag="st")
                    nc.scalar.copy(out=st[:], in_=stps[:])
                    rps = stpool.tile([128, HC], F32, tag="rps")
                    nc.tensor.matmul(out=rps[:], lhsT=st[:], rhs=xr_fl[:],
                                     start=True, stop=True)
                    nc.vector.scalar_tensor_tensor(
                        out=v[:, sg, :], in0=we_rep[:],
                        scalar=mf[:, NSG + sg:NSG + sg + 1], in1=rps[:],
                        op0=ALU.mult, op1=ALU.add,
                    )
                # v += G  (keep the Pool engine clear: SWDGE desc-gen runs
                # there and is the span-wall bottleneck)
                nc.vector.tensor_tensor(out=v[:], in0=v[:], in1=G[:],
                                        op=ALU.add)

                # ---- u = lrelu(v) ; alpha = att . u (fold tree in-place)
                u = pool.tile([128, NSG, HC], F16, tag="u")
                nc.scalar.activation(out=u[:], in_=v[:], func=AF.Lrelu,
                                     alpha=0.2)
                nc.vector.tensor_tensor(
                    out=u[:], in0=u[:],
                    in1=att_rep[:].rearrange("p (o c) -> p o c", o=1)
                    .broadcast_to((128, NSG, HC)),
                    op=ALU.mult)
                u4 = u[:].rearrange("p s (h c) -> p s h c", h=H)
                w = 32
                while w >= 2:
                    nc.vector.tensor_tensor(
                        out=u4[:, :, :, 0:w], in0=u4[:, :, :, 0:w],
                        in1=u4[:, :, :, w:2 * w], op=ALU.add,
                    )
                    w //= 2
                alpha = spool.tile([128, NSG, H], F32, tag="alpha")
                nc.vector.tensor_tensor(
                    out=alpha[:].rearrange("p s (h o) -> p s h o", o=1),
                    in0=u4[:, :, :, 0:1], in1=u4[:, :, :, 1:2], op=ALU.add,
                )

                # ---- m260 = [ex*G | ex] ; fused aggregation matmul
                m260 = pool.tile([128, NSG, 260], F16, tag="m260")
                nc.scalar.activation(out=m260[:, :, 256:260], in_=alpha[:],
                                     func=AF.Exp)
                nc.vector.tensor_tensor(
                    out=m260[:, :, 0:256].rearrange("p s (h c) -> p s h c", h=H),
                    in0=G[:].rearrange("p s (h c) -> p s h c", h=H),
                    in1=m260[:, :, 256:260].rearrange(
                        "p s (h o) -> p s h o", o=1).broadcast_to(
                        (128, NSG, H, C)),
                    op=ALU.mult)
                acc = ppool.tile([128, 260], F32, tag="acc")
                for sg in range(NSG):
                    nc.tensor.matmul(out=acc[:], lhsT=S[:, sg, :],
                                     rhs=m260[:, sg, :], start=(sg == 0),
                                     stop=(sg == NSG - 1))

                # ---- flush: h = relu(accM/den + bias)
                rden = spool.tile([128, 4], F32, tag="rden")
                den = spool.tile([128, 4], F32, tag="den")
                nc.vector.tensor_scalar(
                    out=den[:], in0=acc[:, 256:260], scalar1=1e-30,
                    scalar2=None, op0=ALU.add,
                )
                nc.vector.reciprocal(out=rden[:], in_=den[:])
                hT = spool.tile([128, HC], F16, tag="hT")
                for hh in range(H):
                    blks = slice(hh * C, (hh + 1) * C)
                    nc.vector.scalar_tensor_tensor(
                        out=hT[:, blks], in0=acc[:, blks],
                        scalar=rden[:, hh:hh + 1], in1=bias_rep[:, blks],
                        op0=ALU.mult, op1=ALU.add,
                    )
                hOut = spool.tile([128, HC], F16, tag="hOut")
                nc.scalar.activation(out=hOut[:], in_=hT[:], func=AF.Relu)
                h_sink(s, hOut, mf, pool, spool, ppool, stpool)

    # layer-1 sink: transpose h1 on PE, compute xl2/xr2, write to DRAM
    def h1_sink(s, hOut, mf, pool, spool, ppool, stpool):
        h1T = pool.tile([128, 2, 128], F16, tag="h1T")
        for half in range(2):
            tp = stpool.tile([128, 128], F16, tag="stps")
            nc.tensor.transpose(
                out=tp[:], in_=hOut[:, half * 128:(half + 1) * 128],
                identity=identF[:])
            nc.scalar.copy(out=h1T[:, half, :], in_=tp[:])
        for nm, sink in (("wl_aug", t_xl2_in), ("wr_aug", t_xr2)):
            wa, wb, wc = w2_tiles[nm]
            ps = ppool.tile([128, HC], F32, tag="acc")
            nc.tensor.matmul(out=ps[:], lhsT=h1T[:, 0, :], rhs=wa[:],
                             start=True, stop=False)
            nc.tensor.matmul(out=ps[:], lhsT=h1T[:, 1, :], rhs=wb[:],
                             start=False, stop=False)
            nc.tensor.matmul(out=ps[:], lhsT=ones_col[:], rhs=wc[:],
                             start=False, stop=True)
            xs = spool.tile([128, HC], F16, tag="xs")
            nc.vector.tensor_copy(out=xs[:], in_=ps[:])
            nc.sync.dma_start(out=sink[s * 128:(s + 1) * 128, :], in_=xs[:])

    with tc.tile_pool(name="gpool_ps", bufs=1, space="PSUM") as gpool_ps:
      gpsum = gpool_ps.tile([4, HC], F32)

      if phase_limit >= 2:
          gat_layer(1, t_xl1, t_xr1, h1_sink)

      if phase_limit >= 3:
          # AllGather xl2
          nc.gpsimd.collective_compute(
              "AllGather",
              ALU.bypass,
              replica_groups=[list(range(NCORES))],
              ins=[t_xl2_in.ap().opt()],
              outs=[t_xl2.ap().opt()],
          )

      # layer-2 sink: pooled accumulation (gmask lives in metaF cols 36:40)
      def h2_sink(s, hOut, mf, pool, spool, ppool, stpool):
          nc.tensor.matmul(out=gpsum[:], lhsT=mf[:, 36:40], rhs=hOut[:],
                           start=(s == 0), stop=(s == nspans - 1))

      if phase_limit >= 4:
          gat_layer(2, t_xl2, t_xr2, h2_sink)

      # ------------------------------------------------------------------
      # Pool -> MLP -> out
      # ------------------------------------------------------------------
      if phase_limit >= 5:
          _build_mlp(nc, tc, gpsum, t_inv_cnt, t_p1_aug, t_ln_g, t_ln_b,
                     t_p2_aug, t_out)
      else:
          with tc.tile_pool(name="dummyout", bufs=1) as dpool:
              dz = dpool.tile([4, 64], F32)
              nc.vector.memset(dz[:], 0.0)
              nc.sync.dma_start(out=t_out[:], in_=dz[:])


def _build_mlp(nc, tc, gpsum, t_inv_cnt, t_p1_aug, t_ln_g, t_ln_b, t_p2_aug,
               t_out):
    with tc.tile_pool(name="mlp", bufs=1) as pool, \
         tc.tile_pool(name="mlp_ps", bufs=2, space="PSUM") as ppool:
        icnt = pool.tile([4, 1], F32)
        nc.sync.dma_start(out=icnt[:], in_=t_inv_cnt[:])
        g = pool.tile([4, HC], F32)
        nc.vector.tensor_scalar(out=g[:], in0=gpsum[:], scalar1=icnt[:, 0:1],
                                scalar2=None, op0=ALU.mult)
        p1a = pool.tile([128, 128], F32)
        p1b = pool.tile([128, 128], F32)
        p1c = pool.tile([1, 128], F32)
        nc.sync.dma_start(out=p1a[:], in_=t_p1_aug[0:128, :])
        nc.sync.dma_start(out=p1b[:], in_=t_p1_aug[128:256, :])
        nc.sync.dma_start(out=p1c[:], in_=t_p1_aug[256:257, :])
        p2a = pool.tile([128, 64], F32)
        p2c = pool.tile([1, 64], F32)
        nc.sync.dma_start(out=p2a[:], in_=t_p2_aug[0:128, :])
        nc.sync.dma_start(out=p2c[:], in_=t_p2_aug[128:129, :])
        lng = pool.tile([4, 128], F32)
        nc.sync.dma_start(out=lng[:], in_=t_ln_g[:])
        lnb = pool.tile([4, 128], F32)
        nc.sync.dma_start(out=lnb[:], in_=t_ln_b[:])
        ident = pool.tile([128, 128], F32)
        from concourse.masks import make_identity
        make_identity(nc, ident[:])

        gT = pool.tile([128, 8], F32)
        for half in range(2):
            tp = ppool.tile([128, 128], F32, tag="tp")
            nc.tensor.transpose(
                out=tp[:, 0:4], in_=g[:, half * 128:(half + 1) * 128],
                identity=ident[0:4, 0:4],
            )
            nc.vector.tensor_copy(out=gT[:, half * 4:half * 4 + 4],
                                  in_=tp[:, 0:4])
        onesg = pool.tile([1, 4], F32)
        nc.vector.memset(onesg[:], 1.0)
        z1p = ppool.tile([4, 128], F32, tag="z1p")
        nc.tensor.matmul(out=z1p[:], lhsT=gT[:, 0:4], rhs=p1a[:],
                         start=True, stop=False)
        nc.tensor.matmul(out=z1p[:], lhsT=gT[:, 4:8], rhs=p1b[:],
                         start=False, stop=False)
        nc.tensor.matmul(out=z1p[:], lhsT=onesg[:], rhs=p1c[:],
                         start=False, stop=True)
        z1 = pool.tile([4, 128], F32)
        nc.vector.tensor_copy(out=z1[:], in_=z1p[:])
        mu = pool.tile([4, 1], F32)
        nc.vector.reduce_sum(out=mu[:], in_=z1[:], axis=AXX)
        nc.vector.tensor_scalar(out=mu[:], in0=mu[:], scalar1=1.0 / 128,
                                scalar2=None, op0=ALU.mult)
        zc = pool.tile([4, 128], F32)
        nc.vector.tensor_scalar(out=zc[:], in0=z1[:], scalar1=mu[:, 0:1],
                                scalar2=None, op0=ALU.subtract)
        sq = pool.tile([4, 128], F32)
        nc.vector.tensor_tensor(out=sq[:], in0=zc[:], in1=zc[:], op=ALU.mult)
        var = pool.tile([4, 1], F32)
        nc.vector.reduce_sum(out=var[:], in_=sq[:], axis=AXX)
        nc.vector.tensor_scalar(out=var[:], in0=var[:], scalar1=1.0 / 128,
                                scalar2=1e-5, op0=ALU.mult, op1=ALU.add)
        std = pool.tile([4, 1], F32)
        nc.scalar.activation(out=std[:], in_=var[:], func=AF.Sqrt)
        rstd = pool.tile([4, 1], F32)
        nc.vector.reciprocal(out=rstd[:], in_=std[:])
        zn = pool.tile([4, 128], F32)
        nc.vector.tensor_scalar(out=zn[:], in0=zc[:], scalar1=rstd[:, 0:1],
                                scalar2=None, op0=ALU.mult)
        nc.vector.tensor_tensor(out=zn[:], in0=zn[:], in1=lng[:], op=ALU.mult)
        nc.vector.tensor_tensor(out=zn[:], in0=zn[:], in1=lnb[:], op=ALU.add)
        nc.scalar.activation(out=zn[:], in_=zn[:], func=AF.Relu)
        znT = pool.tile([128, 4], F32)
        tp2 = ppool.tile([128, 128], F32, tag="tp")
        nc.tensor.transpose(out=tp2[:, 0:4], in_=zn[:], identity=ident[0:4, 0:4])
        nc.vector.tensor_copy(out=znT[:], in_=tp2[:, 0:4])
        z2p = ppool.tile([4, 64], F32, tag="z2p")
        nc.tensor.matmul(out=z2p[:], lhsT=znT[:], rhs=p2a[:],
                         start=True, stop=False)
        nc.tensor.matmul(out=z2p[:], lhsT=onesg[:], rhs=p2c[:],
                         start=False, stop=True)
        zout = pool.tile([4, 64], F32)
        nc.scalar.activation(out=zout[:], in_=z2p[:], func=AF.Relu)
        nc.sync.dma_start(out=t_out[:], in_=zout[:])


# ----------------------------------------------------------------------------
# Entry point
# ----------------------------------------------------------------------------

def _pack_inputs(inp, cores, packs, nspans, rows_per_core, rows_total, x_aug_T):
    f16 = np.float16
    iota_row = np.broadcast_to(
        np.arange(128, dtype=f16)[None, :], (128, 128)
    ).copy()
    in_maps = []
    for k in range(NCORES):
        p = packs[k]
        m = {
            "xaugT": x_aug_T.astype(np.float32),
            "own_xaugT": np.ascontiguousarray(
                x_aug_T[:, p["own_cols"]]
            ).astype(np.float32),
            "enc_aug": np.concatenate(
                [np.asarray(inp["enc_w"], np.float32),
                 np.asarray(inp["enc_b"], np.float32)[None, :]], 0
            ),
            "idxsw": p["idxsw"],
            "idxqp": p["idxqp"],
            "swcnt": p["swcnt"],
            "metaF": p["metaF"],
            "iota_row": iota_row,
            "inv_cnt": p["inv_cnt"],
            "p1_aug": np.concatenate(
                [np.asarray(inp["p1_w"], np.float32),
                 np.asarray(inp["p1_b"], np.float32)[None, :]], 0
            ),
            "ln_g4": np.broadcast_to(
                np.asarray(inp["ln_g"], np.float32)[None, :], (4, 128)
            ).copy(),
            "ln_b4": np.broadcast_to(
                np.asarray(inp["ln_b"], np.float32)[None, :], (4, 128)
            ).copy(),
            "p2_aug": np.concatenate(
                [np.asarray(inp["p2_w"], np.float32),
                 np.asarray(inp["p2_b"], np.float32)[None, :]], 0
            ),
        }
        for L in (1, 2):
            wl = np.asarray(inp[f"g{L}_wl"], np.float32)
            bl = np.asarray(inp[f"g{L}_bl"], np.float32)
            wr = np.asarray(inp[f"g{L}_wr"], np.float32)
            br = np.asarray(inp[f"g{L}_br"], np.float32)
            bias = np.asarray(inp[f"g{L}_bias"], np.float32)
            m[f"wl{L}_aug"] = np.concatenate([wl, bl[None, :]], 0).astype(f16)
            m[f"wr{L}_aug"] = np.concatenate([wr, br[None, :]], 0).astype(f16)
            m[f"bias{L}_row"] = np.broadcast_to(
                bias.reshape(1, HC), (128, HC)
            ).astype(f16).copy()
            m[f"att{L}_row"] = np.broadcast_to(
                np.asarray(inp[f"g{L}_att"], np.float32).reshape(1, HC), (128, HC)
            ).astype(f16).copy()
            m[f"we{L}_row"] = np.broadcast_to(
                np.asarray(inp[f"g{L}_we"], np.float32).reshape(1, HC), (128, HC)
            ).astype(f16).copy()
        in_maps.append(m)
    return in_maps


def kernel(**inputs):
    cores, packs, nspans, rows_per_core, rows_total, x_aug_T, node_row = _host_prep(
        inputs
    )
    key = (nspans, rows_total)
    if key not in _PROGRAM_CACHE:
        _PROGRAM_CACHE[key] = _build_program(nspans, rows_total)
    nc = _PROGRAM_CACHE[key]
    in_maps = _pack_inputs(
        inputs, cores, packs, nspans, rows_per_core, rows_total, x_aug_T
    )
    res = run_bass_kernel_spmd(nc, in_maps, core_ids=list(range(NCORES)))
    out = np.concatenate([res.results[k]["out"] for k in range(NCORES)], axis=0)
    return out.astype(np.float32)


if __name__ == "__main__":
    data = dict(np.load("/root/problem/inputs_cache.npz"))
    out = kernel(**data)
    exp = np.load("/root/problem/expected_np.npy")
    rel = np.linalg.norm(out - exp) / np.linalg.norm(exp)
    print("rel err:", rel)


# revision 3
# speedup vs baseline: 2.1196x; 2.1196x over previous
"""Trainium2 Bass kernel for nn_GATv2Base (gnn_message_passing) — v2.

Contract: kernel(**inputs) takes FULL unsharded inputs (same keys as
reference.setup_inputs()) and returns the FULL [32, 64] float32 output.

Sharding: 32 graphs -> 8 cores (4 graphs each, contiguous node ranges since
`batch` is sorted).  Edges (plus self-loops) are routed to the core owning
their dst node, sorted by dst, and packed into spans (<=127-node dst window,
2304 edge slots = 18 subgroups of 128; subgroups 0-8 hold edges whose src row
is in the lower half of the padded node table, 9-17 the upper half).

Per span, the source-feature gather xl[src] runs on BOTH descriptor-generation
engines in parallel: the first 7 subgroups of each half via SWDGE dma_gather
(4 queues, int16 half-relative indices), the last 2 via hardware-DGE
indirect_dma_start (int32 global indices).  The dst-side term xr[dst] is NOT
gathered: each dst row lives in the span's own 128-row window, so it is
expanded edge-wise on the PE with one-hot matrices built from the dst column
(S for the scatter direction, S^T via PE transpose for the broadcast
direction).

Layer 1 output h1 never goes to DRAM: the span sink transposes it on the PE
and immediately computes xl2/xr2.  One fp16 AllGather shares the xl2 table;
the pooled per-graph MLP runs on the owning core.  Only [4, 64] per-core
outputs return to the host.
"""

import os
import sys

import numpy as np

for _p in ("/opt/trn_rl_repo", "/root/.axon_site/_ro/trn_rl_repo"):
    if os.path.isdir(_p) and _p not in sys.path:
        sys.path.insert(0, _p)

import concourse.bass as bass
import concourse.bacc as bacc
import concourse.mybir as mybir
import concourse.tile as tile
from concourse.bass import IndirectOffsetOnAxis
from concourse.bass_utils import run_bass_kernel_spmd
from concourse.masks import make_identity

F32 = mybir.dt.float32
F16 = mybir.dt.float16
I32 = mybir.dt.int32
I16 = mybir.dt.int16
AF = mybir.ActivationFunctionType
ALU = mybir.AluOpType
AXX = mybir.AxisListType.X

N, E, H, C, NG = 50000, 800000, 4, 64, 32
HC = H * C
NCORES = 8
SLOT_H = 1152            # edge slots per table-half region (9 subgroups)
SPAN_EDGES = 2 * SLOT_H  # 2304 edge slots per span
NSG = SPAN_EDGES // 128  # 18
HSG = NSG // 2           # 9
NQP = 2                  # subgroups per half gathered via qPoolDynamic
NSW = HSG - NQP          # subgroups per half gathered via SWDGE
SPAN_DST = 127           # dst window per span; 127 = pad marker
_PAD0 = True             # pad swdge idx with 0 (-1 auto-trim crashes this ucode)
_REGTRIM = False         # reg/-1 desc trimming both crash this ucode build


# ----------------------------------------------------------------------------
# Host-side sharding / packing
# ----------------------------------------------------------------------------

def _host_prep(inp):
    x = np.asarray(inp["x"], dtype=np.float32)
    ei = np.asarray(inp["edge_index"], dtype=np.int32)
    ea_full = np.asarray(inp["edge_attr"], dtype=np.float32)[:, 0]
    batch = np.asarray(inp["batch"], dtype=np.int32)

    src0, dst0 = ei[0], ei[1]
    deg = np.maximum(np.bincount(dst0, minlength=N).astype(np.float64), 1.0)
    loop_attr = (
        np.bincount(dst0, weights=ea_full.astype(np.float64), minlength=N) / deg
    ).astype(np.float32)
    src = np.concatenate([src0, np.arange(N, dtype=np.int32)])
    dst = np.concatenate([dst0, np.arange(N, dtype=np.int32)])
    eattr = np.concatenate([ea_full, loop_attr]).astype(np.float32)

    gcounts = np.bincount(batch, minlength=NG)
    gstart = np.concatenate([[0], np.cumsum(gcounts)])
    core_n0 = np.array([gstart[4 * k] for k in range(NCORES)] + [N], dtype=np.int64)

    order = np.argsort(dst, kind="stable")
    src, dst, eattr = src[order], dst[order], eattr[order]
    edge_lo = np.searchsorted(dst, core_n0[:-1], "left")
    edge_hi = np.searchsorted(dst, core_n0[1:], "left")

    src_owner = np.searchsorted(core_n0[1:], src, "right")
    src_in_a = src_owner < (NCORES // 2)

    cores = []
    for k in range(NCORES):
        n0, n1 = int(core_n0[k]), int(core_n0[k + 1])
        s, e = int(edge_lo[k]), int(edge_hi[k])
        cd = dst[s:e]
        ca = src_in_a[s:e]
        nlocal = n1 - n0
        node_edge_start = np.searchsorted(cd, n0 + np.arange(nlocal + 1))
        cumA = np.concatenate([[0], np.cumsum(ca)])
        spans = []
        b = 0
        while b < nlocal:
            bend = b
            while bend < nlocal and (bend - b) < SPAN_DST:
                e0, e1 = node_edge_start[b], node_edge_start[bend + 1]
                nA = cumA[e1] - cumA[e0]
                nB = (e1 - e0) - nA
                if nA > SLOT_H or nB > SLOT_H:
                    break
                bend += 1
            assert bend > b, "single node exceeds span edge capacity"
            spans.append(
                (b, bend - b, int(node_edge_start[b]), int(node_edge_start[bend]))
            )
            b = bend
        cores.append(
            dict(n0=n0, n1=n1, spans=spans, src=src[s:e], dst=cd, ea=eattr[s:e],
                 in_a=ca)
        )

    nspans = max(len(c["spans"]) for c in cores)
    rows_per_core = nspans * 128
    rows_total = NCORES * rows_per_core
    half_rows = rows_total // 2
    assert half_rows <= 32767, f"table half {half_rows} exceeds int16 index range"

    node_row = np.zeros(N, dtype=np.int64)
    for k, c in enumerate(cores):
        for si, (b, nb, _, _) in enumerate(c["spans"]):
            nodes = np.arange(c["n0"] + b, c["n0"] + b + nb)
            node_row[nodes] = k * rows_per_core + si * 128 + (nodes - c["n0"] - b)

    x_pad = np.zeros((rows_total, 4), dtype=np.float32)
    x_pad[node_row] = x
    x_aug_T = np.concatenate(
        [x_pad.T, np.ones((1, rows_total), dtype=np.float32)], axis=0
    )  # [5, R]

    def wrap16(vals):
        # [n] -> wrapped [128, n//16] int16 (16-part wrap, replicated x8)
        n = len(vals)
        base = np.zeros((16, n // 16), dtype=np.int16)
        i = np.arange(n)
        base[i % 16, i // 16] = vals.astype(np.int16)
        return np.tile(base, (8, 1))

    sw_cols = NSW * 128 // 16  # idx cols per half for the SWDGE portion
    packs = []
    for k, c in enumerate(cores):
        idxsw = np.zeros((nspans, 128, 2, sw_cols), np.int16)
        idxqp = np.zeros((nspans, 128, 2 * NQP), np.int32)
        swcnt = np.full((nspans, 1, 4), 16, np.int32)
        metaF = np.zeros((nspans, 128, 40), np.float16)
        metaF[:, :, 0:NSG] = 127.0  # dcol pad marker
        for si, (b, nb, e0, e1) in enumerate(c["spans"]):
            ina = c["in_a"][e0:e1]
            esrc = node_row[c["src"][e0:e1]]
            edrel = (c["dst"][e0:e1] - c["n0"] - b).astype(np.int64)
            eea = c["ea"][e0:e1]
            # slots: A edges at [0, SLOT_H), B edges at [SLOT_H, 2*SLOT_H)
            ia = np.where(ina)[0]
            ib = np.where(~ina)[0]
            slots = np.empty(len(ina), dtype=np.int64)
            slots[ia] = np.arange(len(ia))
            slots[ib] = SLOT_H + np.arange(len(ib))
            for h, iv in ((0, ia), (1, ib)):
                # qpool takes the FIRST NQP subgroups (always full of real
                # edges), swdge the rest: trailing pads are marked -1 so the
                # q7 desc-gen kernel auto-trims them.
                rows_h = np.full(SLOT_H, -1, dtype=np.int64)
                rows_h[:len(iv)] = esrc[iv] - h * half_rows
                qp = rows_h[:NQP * 128] + h * half_rows  # global rows
                qp[qp < 0] = 0
                idxqp[si, :, h * NQP:(h + 1) * NQP] = qp.reshape(NQP, 128).T
                sw = rows_h[NQP * 128:]
                sw[sw < 0] = 0 if _PAD0 else -1
                idxsw[si, :, h, :] = wrap16(sw)
                hs_ = (HSG - NQP) // 2
                nH = len(iv)
                swcnt[si, 0, 2 * h + 0] = np.clip(nH - NQP * 128, 16, hs_ * 128)
                swcnt[si, 0, 2 * h + 1] = np.clip(
                    nH - (NQP + hs_) * 128, 16, (HSG - NQP - hs_) * 128)
            p, sg = slots % 128, slots // 128
            metaF[si, p, sg] = edrel.astype(np.float16)            # dcol
            metaF[si, p, NSG + sg] = eea.astype(np.float16)        # eac
            nodes = np.arange(c["n0"] + b, c["n0"] + b + nb)
            gl = batch[nodes] - 4 * k
            metaF[si, np.arange(nb), 2 * NSG + gl] = np.float16(1.0)  # gmask
        inv_cnt = np.zeros((4, 1), dtype=np.float32)
        for gg in range(4):
            cnt = max(int(gcounts[4 * k + gg]), 1)
            inv_cnt[gg, 0] = 1.0 / cnt
        packs.append(
            dict(
                idxsw=idxsw,
                idxqp=idxqp,
                swcnt=swcnt,
                metaF=metaF,
                inv_cnt=inv_cnt,
                own_cols=np.arange(
                    k * rows_per_core, (k + 1) * rows_per_core, dtype=np.int64
                ),
            )
        )
    return cores, packs, nspans, rows_per_core, rows_total, x_aug_T, node_row


# ----------------------------------------------------------------------------
# Device program
# ----------------------------------------------------------------------------

_PROGRAM_CACHE = {}


def _build_program(nspans, rows_total, repeat=1, phase_limit=9):
    rows_per_core = nspans * 128
    nblocks = rows_total // 128
    half_rows = rows_total // 2
    sw_cols = NSW * 128 // 16

    nc = bacc.Bacc(num_swdge_queues=4)
    tcx = tile.TileContext(nc)

    def din(name, shape, dt):
        return nc.dram_tensor(name, shape, dt, kind="ExternalInput")

    t_xaugT = din("xaugT", [5, rows_total], F32)
    t_own_xaugT = din("own_xaugT", [5, rows_per_core], F32)
    t_enc_aug = din("enc_aug", [5, 64], F32)
    t_w1 = {}
    t_w2 = {}
    for L, tw in ((1, t_w1), (2, t_w2)):
        kdim = 65 if L == 1 else 257
        tw["wl_aug"] = din(f"wl{L}_aug", [kdim, HC], F16)
        tw["wr_aug"] = din(f"wr{L}_aug", [kdim, HC], F16)
        tw["att_row"] = din(f"att{L}_row", [128, HC], F16)
        tw["we_row"] = din(f"we{L}_row", [128, HC], F16)
        tw["bias_row"] = din(f"bias{L}_row", [128, HC], F16)
    t_idxsw = din("idxsw", [nspans, 128, 2, sw_cols], I16)
    t_idxqp = din("idxqp", [nspans, 128, 2 * NQP], I32)
    t_swcnt = din("swcnt", [nspans, 1, 4], I32)
    t_metaF = din("metaF", [nspans, 128, 40], F16)
    t_iota_row = din("iota_row", [128, 128], F16)
    t_inv_cnt = din("inv_cnt", [4, 1], F32)
    t_p1_aug = din("p1_aug", [257, 128], F32)
    t_ln_g = din("ln_g4", [4, 128], F32)
    t_ln_b = din("ln_b4", [4, 128], F32)
    t_p2_aug = din("p2_aug", [129, 64], F32)
    t_out = nc.dram_tensor("out", [4, 64], F32, kind="ExternalOutput")

    # internal DRAM
    t_xl1 = nc.dram_tensor("xl1_tbl", [rows_total, HC], F16)
    t_xr1 = nc.dram_tensor("xr1_own", [rows_per_core, HC], F16)
    t_xr2 = nc.dram_tensor("xr2_own", [rows_per_core, HC], F16)
    t_xl2_in = nc.dram_tensor("xl2_own_cc", [rows_per_core, HC], F16)
    t_xl2 = nc.dram_tensor("xl2_tbl", [rows_total, HC], F16, addr_space="Shared")

    from contextlib import ExitStack
    with tcx as tc, ExitStack() as es:
        # constants
        cpool = es.enter_context(tc.tile_pool(name="consts", bufs=1))
        enc_aug = cpool.tile([5, 64], F32)
        nc.sync.dma_start(out=enc_aug[:], in_=t_enc_aug[:])
        iota_rep = cpool.tile([128, 128], F16)
        nc.sync.dma_start(out=iota_rep[:], in_=t_iota_row[:])
        reps = {}
        for L, tw in ((1, t_w1), (2, t_w2)):
            for nm in ("att_row", "we_row", "bias_row"):
                rep = cpool.tile([128, HC], F16, tag=f"rep{L}{nm}")
                nc.sync.dma_start(out=rep[:], in_=tw[nm][:])
                reps[(L, nm)] = rep
        ones_col = cpool.tile([1, 128], F16)
        nc.vector.memset(ones_col[:], 1.0)
        identF = cpool.tile([128, 128], F16)
        make_identity(nc, identF[:])

        wpool = es.enter_context(tc.tile_pool(name="weights", bufs=1))
        wl1 = wpool.tile([65, HC], F16)
        wr1 = wpool.tile([65, HC], F16)
        nc.sync.dma_start(out=wl1[:], in_=t_w1["wl_aug"][:])
        nc.sync.dma_start(out=wr1[:], in_=t_w1["wr_aug"][:])
        w2_tiles = {}
        for nm in ("wl_aug", "wr_aug"):
            a = wpool.tile([128, HC], F16, tag=f"{nm}a")
            b = wpool.tile([128, HC], F16, tag=f"{nm}b")
            cb = wpool.tile([1, HC], F16, tag=f"{nm}c")
            nc.sync.dma_start(out=a[:], in_=t_w2[nm][0:128, :])
            nc.sync.dma_start(out=b[:], in_=t_w2[nm][128:256, :])
            nc.sync.dma_start(out=cb[:], in_=t_w2[nm][256:257, :])
            w2_tiles[nm] = (a, b, cb)

        for _rep in range(repeat):
            _build_iteration(
                nc, tc, nspans, rows_total, rows_per_core, nblocks, half_rows,
                sw_cols, enc_aug, iota_rep, identF, reps, ones_col, wl1, wr1,
                w2_tiles,
                t_xaugT, t_own_xaugT, t_idxsw, t_idxqp, t_swcnt, t_metaF,
                t_inv_cnt,
                t_p1_aug, t_ln_g, t_ln_b, t_p2_aug, t_out,
                t_xl1, t_xr1, t_xr2, t_xl2_in, t_xl2, phase_limit,
            )

    nc.finalize()
    return nc


def _build_iteration(
    nc, tc, nspans, rows_total, rows_per_core, nblocks, half_rows, sw_cols,
    enc_aug, iota_rep, identF, reps, ones_col, wl1, wr1, w2_tiles,
    t_xaugT, t_own_xaugT, t_idxsw, t_idxqp, t_swcnt, t_metaF, t_inv_cnt,
    t_p1_aug, t_ln_g, t_ln_b, t_p2_aug, t_out,
    t_xl1, t_xr1, t_xr2, t_xl2_in, t_xl2, phase_limit=9,
):
    swregs = None
    if _REGTRIM:
        swregs = []
        for j in range(4):
            _swreg = nc.alloc_register(
                mybir.EngineType.Pool, f"swcnt{j}_{nc.next_id()}")
            swregs.append(_swreg)
    # ------------------------------------------------------------------
    # Phase 1: encoder + xl1 for ALL rows (replicated) + own xr1
    # ------------------------------------------------------------------
    def encode_block(pool, ppool, xaugT_ap):
        xT = pool.tile([5, 128], F32, tag="xT")
        nc.sync.dma_start(out=xT[:], in_=xaugT_ap)
        h0psum = ppool.tile([64, 128], F32, tag="h0ps")
        nc.tensor.matmul(out=h0psum[:], lhsT=enc_aug[:], rhs=xT[:],
                         start=True, stop=True)
        h0T = pool.tile([65, 128], F16, tag="h0T")
        nc.scalar.activation(out=h0T[0:64, :], in_=h0psum[:], func=AF.Relu)
        nc.vector.tensor_copy(out=h0T[64:65, :], in_=ones_col[:])
        return h0T

    with tc.tile_pool(name="p1", bufs=3) as pool, \
         tc.tile_pool(name="p1ps", bufs=2, space="PSUM") as ppool:
        for blk in range(nblocks):
            h0T = encode_block(pool, ppool, t_xaugT[:, blk * 128:(blk + 1) * 128])
            xlp = ppool.tile([128, HC], F32, tag="xlps")
            nc.tensor.matmul(out=xlp[:], lhsT=h0T[:], rhs=wl1[:],
                             start=True, stop=True)
            xls = pool.tile([128, HC], F16, tag="xls")
            if blk % 2 == 0:
                nc.vector.tensor_copy(out=xls[:], in_=xlp[:])
            else:
                nc.scalar.copy(out=xls[:], in_=xlp[:])
            nc.sync.dma_start(
                out=t_xl1[blk * 128:(blk + 1) * 128, :], in_=xls[:]
            )
        for s in range(nspans):
            h0T = encode_block(pool, ppool, t_own_xaugT[:, s * 128:(s + 1) * 128])
            xrp = ppool.tile([128, HC], F32, tag="xlps")
            nc.tensor.matmul(out=xrp[:], lhsT=h0T[:], rhs=wr1[:],
                             start=True, stop=True)
            xrs = pool.tile([128, HC], F16, tag="xls")
            nc.vector.tensor_copy(out=xrs[:], in_=xrp[:])
            nc.sync.dma_start(
                out=t_xr1[s * 128:(s + 1) * 128, :], in_=xrs[:]
            )

    # ------------------------------------------------------------------
    # GAT span loop (both layers)
    # ------------------------------------------------------------------
    def gat_layer(L, xl_tbl, xr_tbl, h_sink):
        att_rep = reps[(L, "att_row")]
        we_rep = reps[(L, "we_row")]
        bias_rep = reps[(L, "bias_row")]
        with tc.tile_pool(name=f"g{L}", bufs=2) as pool, \
             tc.tile_pool(name=f"g{L}g", bufs=3) as gqpool, \
             tc.tile_pool(name=f"g{L}b", bufs=3) as spool, \
             tc.tile_pool(name=f"g{L}st", bufs=2, space="PSUM") as stpool, \
             tc.tile_pool(name=f"g{L}ps", bufs=2, space="PSUM") as ppool:
            for s in range(nspans):
                iw = spool.tile([128, 2, sw_cols], I16, tag="iw")
                nc.sync.dma_start(out=iw[:], in_=t_idxsw[s])
                iq = spool.tile([128, 2 * NQP], I32, tag="iq")
                nc.sync.dma_start(out=iq[:], in_=t_idxqp[s])
                mf = spool.tile([128, 40], F16, tag="mf")
                nc.sync.dma_start(out=mf[:], in_=t_metaF[s])
                if _REGTRIM:
                    cnt = spool.tile([1, 4], I32, tag="cnt")
                    nc.sync.dma_start(out=cnt[:], in_=t_swcnt[s])
                xr_fl = spool.tile([128, HC], F16, tag="xrfl")
                nc.sync.dma_start(
                    out=xr_fl[:], in_=xr_tbl[s * 128:(s + 1) * 128, :]
                )

                # ---- G = xl[src] gather: qPoolDynamic (first NQP subgroups
                # per half) + SWDGE on 4 queues (rest; trailing pads trimmed)
                G = gqpool.tile([128, NSG, HC], F16, tag="G")
                if s < 3:
                    # pad slots trimmed from the gather leave stale SBUF
                    # bytes; seed the two ring buffers once so they are
                    # always finite.
                    nc.vector.memset(G[:].rearrange("p a b -> p (a b)"), 0.0)
                hs = NSW // 2  # 3|4 split of the SWDGE subgroups
                for h in range(2):
                    base = h * HSG
                    tbl_half = (xl_tbl[0:half_rows, :] if h == 0
                                else xl_tbl[half_rows:, :])
                    for j in range(NQP):
                        nc.gpsimd.indirect_dma_start(
                            out=G[:, base + j, :],
                            out_offset=None,
                            in_=xl_tbl[:],
                            in_offset=IndirectOffsetOnAxis(
                                ap=iq[:, h * NQP + j:h * NQP + j + 1], axis=0),
                        )
                    if _REGTRIM:
                        nc.gpsimd.reg_load(swregs[2 * h + 0],
                                           cnt[0:1, 2 * h + 0:2 * h + 1])
                        nc.gpsimd.reg_load(swregs[2 * h + 1],
                                           cnt[0:1, 2 * h + 1:2 * h + 2])
                    r0 = swregs[2 * h + 0] if _REGTRIM else hs * 128
                    r1 = swregs[2 * h + 1] if _REGTRIM else (NSW - hs) * 128
                    nc.gpsimd.dma_gather(
                        G[:, base + NQP:base + NQP + hs, :], tbl_half,
                        iw[:, h, 0:hs * 8], hs * 128, r0, HC,
                        single_packet=False, queue_num=2 * h)
                    nc.gpsimd.dma_gather(
                        G[:, base + NQP + hs:base + HSG, :], tbl_half,
                        iw[:, h, hs * 8:NSW * 8], (NSW - hs) * 128,
                        r1, HC,
                        single_packet=False, queue_num=2 * h + 1)

                # ---- S one-hot [e_part, sg, d] (one fused DVE op)
                S = pool.tile([128, NSG, 128], F16, tag="S")
                dco = mf[:, 0:NSG].rearrange(
                    "p (a o) -> p a o", o=1).broadcast_to((128, NSG, 128))
                iot = iota_rep[:].rearrange(
                    "p (o c) -> p o c", o=1).broadcast_to((128, NSG, 128))
                nc.vector.tensor_tensor(out=S[:], in0=dco, in1=iot,
                                        op=ALU.is_equal)

                # ---- R expansion + v = we*ea + R   (per subgroup)
                v = pool.tile([128, NSG, HC], F16, tag="v")
                for sg in range(NSG):
                    stps = stpool.tile([128, 128], F16, tag="stps")
                    nc.tensor.transpose(out=stps[:], in_=S[:, sg, :],
                                        identity=identF[:])
                    st = pool.tile([128, 128], F16, tag="st")
                    nc.scalar.copy(out=st[:], in_=stps[:])
                    rps = stpool.tile([128, HC], F32, tag="rps")
                    nc.tensor.matmul(out=rps[:], lhsT=st[:], rhs=xr_fl[:],
                                     start=True, stop=True)
                    nc.vector.scalar_tensor_tensor(
                        out=v[:, sg, :], in0=we_rep[:],
                        scalar=mf[:, NSG + sg:NSG + sg + 1], in1=rps[:],
                        op0=ALU.mult, op1=ALU.add,
                    )
                # v += G  (keep the Pool engine clear: SWDGE desc-gen runs
                # there and is the span-wall bottleneck)
                nc.vector.tensor_tensor(out=v[:], in0=v[:], in1=G[:],
                                        op=ALU.add)

                # ---- u = lrelu(v) ; alpha = att . u (fold tree in-place)
                u = pool.tile([128, NSG, HC], F16, tag="u")
                nc.scalar.activation(out=u[:], in_=v[:], func=AF.Lrelu,
                                     alpha=0.2)
                nc.vector.tensor_tensor(
                    out=u[:], in0=u[:],
                    in1=att_rep[:].rearrange("p (o c) -> p o c", o=1)
                    .broadcast_to((128, NSG, HC)),
                    op=ALU.mult)
                u4 = u[:].rearrange("p s (h c) -> p s h c", h=H)
                w = 32
                while w >= 2:
                    nc.vector.tensor_tensor(
                        out=u4[:, :, :, 0:w], in0=u4[:, :, :, 0:w],
                        in1=u4[:, :, :, w:2 * w], op=ALU.add,
                    )
                    w //= 2
                alpha = spool.tile([128, NSG, H], F32, tag="alpha")
                nc.vector.tensor_tensor(
                    out=alpha[:].rearrange("p s (h o) -> p s h o", o=1),
                    in0=u4[:, :, :, 0:1], in1=u4[:, :, :, 1:2], op=ALU.add,
                )

                # ---- m260 = [ex*G | ex] ; fused aggregation matmul
                m260 = pool.tile([128, NSG, 260], F16, tag="m260")
                nc.scalar.activation(out=m260[:, :, 256:260], in_=alpha[:],
                                     func=AF.Exp)
                nc.vector.tensor_tensor(
                    out=m260[:, :, 0:256].rearrange("p s (h c) -> p s h c", h=H),
                    in0=G[:].rearrange("p s (h c) -> p s h c", h=H),
                    in1=m260[:, :, 256:260].rearrange(
                        "p s (h o) -> p s h o", o=1).broadcast_to(
                        (128, NSG, H, C)),
                    op=ALU.mult)
                acc = ppool.tile([128, 260], F32, tag="acc")
                for sg in range(NSG):
                    nc.tensor.matmul(out=acc[:], lhsT=S[:, sg, :],
                                     rhs=m260[:, sg, :], start=(sg == 0),
                                     stop=(sg == NSG - 1))

                # ---- flush: h = relu(accM/den + bias)
                rden = spool.tile([128, 4], F32, tag="rden")
                den = spool.tile([128, 4], F32, tag="den")
                nc.vector.tensor_scalar(
                    out=den[:], in0=acc[:, 256:260], scalar1=1e-30,
                    scalar2=None, op0=ALU.add,
                )
                nc.vector.reciprocal(out=rden[:], in_=den[:])
                hT = spool.tile([128, HC], F16, tag="hT")
                for hh in range(H):
                    blks = slice(hh * C, (hh + 1) * C)
                    nc.vector.scalar_tensor_tensor(
                        out=hT[:, blks], in0=acc[:, blks],
                        scalar=rden[:, hh:hh + 1], in1=bias_rep[:, blks],
                        op0=ALU.mult, op1=ALU.add,
                    )
                hOut = spool.tile([128, HC], F16, tag="hOut")
                nc.scalar.activation(out=hOut[:], in_=hT[:], func=AF.Relu)
                h_sink(s, hOut, mf, pool, spool, ppool, stpool)

    # layer-1 sink: transpose h1 on PE, compute xl2/xr2, write to DRAM
    def h1_sink(s, hOut, mf, pool, spool, ppool, stpool):
        h1T = pool.tile([128, 2, 128], F16, tag="h1T")
        for half in range(2):
            tp = stpool.tile([128, 128], F16, tag="stps")
            nc.tensor.transpose(
                out=tp[:], in_=hOut[:, half * 128:(half + 1) * 128],
                identity=identF[:])
            nc.scalar.copy(out=h1T[:, half, :], in_=tp[:])
        for nm, sink in (("wl_aug", t_xl2_in), ("wr_aug", t_xr2)):
            wa, wb, wc = w2_tiles[nm]
            ps = ppool.tile([128, HC], F32, tag="acc")
            nc.tensor.matmul(out=ps[:], lhsT=h1T[:, 0, :], rhs=wa[:],
                             start=True, stop=False)
            nc.tensor.matmul(out=ps[:], lhsT=h1T[:, 1, :], rhs=wb[:],
                             start=False, stop=False)
            nc.tensor.matmul(out=ps[:], lhsT=ones_col[:], rhs=wc[:],
                             start=False, stop=True)
            xs = spool.tile([128, HC], F16, tag="xs")
            nc.vector.tensor_copy(out=xs[:], in_=ps[:])
            nc.sync.dma_start(out=sink[s * 128:(s + 1) * 128, :], in_=xs[:])

    with tc.tile_pool(name="gpool_ps", bufs=1, space="PSUM") as gpool_ps:
      gpsum = gpool_ps.tile([4, HC], F32)

      if phase_limit >= 2:
          gat_layer(1, t_xl1, t_xr1, h1_sink)

      if phase_limit >= 3:
          # AllGather xl2
          nc.gpsimd.collective_compute(
              "AllGather",
              ALU.bypass,
              replica_groups=[list(range(NCORES))],
              ins=[t_xl2_in.ap().opt()],
              outs=[t_xl2.ap().opt()],
          )

      # layer-2 sink: pooled accumulation (gmask lives in metaF cols 36:40)
      def h2_sink(s, hOut, mf, pool, spool, ppool, stpool):
          nc.tensor.matmul(out=gpsum[:], lhsT=mf[:, 36:40], rhs=hOut[:],
                           start=(s == 0), stop=(s == nspans - 1))

      if phase_limit >= 4:
          gat_layer(2, t_xl2, t_xr2, h2_sink)

      # ------------------------------------------------------------------
      # Pool -> MLP -> out
      # ------------------------------------------------------------------
      if phase_limit >= 5:
          _build_mlp(nc, tc, gpsum, t_inv_cnt, t_p1_aug, t_ln_g, t_ln_b,
                     t_p2_aug, t_out)
      else:
          with tc.tile_pool(name="dummyout", bufs=1) as dpool:
              dz = dpool.tile([4, 64], F32)
              nc.vector.memset(dz[:], 0.0)
              nc.sync.dma_start(out=t_out[:], in_=dz[:])


def _build_mlp(nc, tc, gpsum, t_inv_cnt, t_p1_aug, t_ln_g, t_ln_b, t_p2_aug,
               t_out):
    with tc.tile_pool(name="mlp", bufs=1) as pool, \
         tc.tile_pool(name="mlp_ps", bufs=2, space="PSUM") as ppool:
        icnt = pool.tile([4, 1], F32)
        nc.sync.dma_start(out=icnt[:], in_=t_inv_cnt[:])
        g = pool.tile([4, HC], F32)
        nc.vector.tensor_scalar(out=g[:], in0=gpsum[:], scalar1=icnt[:, 0:1],
                                scalar2=None, op0=ALU.mult)
        p1a = pool.tile([128, 128], F32)
        p1b = pool.tile([128, 128], F32)
        p1c = pool.tile([1, 128], F32)
        nc.sync.dma_start(out=p1a[:], in_=t_p1_aug[0:128, :])
        nc.sync.dma_start(out=p1b[:], in_=t_p1_aug[128:256, :])
        nc.sync.dma_start(out=p1c[:], in_=t_p1_aug[256:257, :])
        p2a = pool.tile([128, 64], F32)
        p2c = pool.tile([1, 64], F32)
        nc.sync.dma_start(out=p2a[:], in_=t_p2_aug[0:128, :])
        nc.sync.dma_start(out=p2c[:], in_=t_p2_aug[128:129, :])
        lng = pool.tile([4, 128], F32)
        nc.sync.dma_start(out=lng[:], in_=t_ln_g[:])
        lnb = pool.tile([4, 128], F32)
        nc.sync.dma_start(out=lnb[:], in_=t_ln_b[:])
        ident = pool.tile([128, 128], F32)
        from concourse.masks import make_identity
        make_identity(nc, ident[:])

        gT = pool.tile([128, 8], F32)
        for half in range(2):
            tp = ppool.tile([128, 128], F32, tag="tp")
            nc.tensor.transpose(
                out=tp[:, 0:4], in_=g[:, half * 128:(half + 1) * 128],
                identity=ident[0:4, 0:4],
            )
            nc.vector.tensor_copy(out=gT[:, half * 4:half * 4 + 4],
                                  in_=tp[:, 0:4])
        onesg = pool.tile([1, 4], F32)
        nc.vector.memset(onesg[:], 1.0)
        z1p = ppool.tile([4, 128], F32, tag="z1p")
        nc.tensor.matmul(out=z1p[:], lhsT=gT[:, 0:4], rhs=p1a[:],
                         start=True, stop=False)
        nc.tensor.matmul(out=z1p[:], lhsT=gT[:, 4:8], rhs=p1b[:],
                         start=False, stop=False)
        nc.tensor.matmul(out=z1p[:], lhsT=onesg[:], rhs=p1c[:],
                         start=False, stop=True)
        z1 = pool.tile([4, 128], F32)
        nc.vector.tensor_copy(out=z1[:], in_=z1p[:])
        mu = pool.tile([4, 1], F32)
        nc.vector.reduce_sum(out=mu[:], in_=z1[:], axis=AXX)
        nc.vector.tensor_scalar(out=mu[:], in0=mu[:], scalar1=1.0 / 128,
                                scalar2=None, op0=ALU.mult)
        zc = pool.tile([4, 128], F32)
        nc.vector.tensor_scalar(out=zc[:], in0=z1[:], scalar1=mu[:, 0:1],
                                scalar2=None, op0=ALU.subtract)
        sq = pool.tile([4, 128], F32)
        nc.vector.tensor_tensor(out=sq[:], in0=zc[:], in1=zc[:], op=ALU.mult)
        var = pool.tile([4, 1], F32)
        nc.vector.reduce_sum(out=var[:], in_=sq[:], axis=AXX)
        nc.vector.tensor_scalar(out=var[:], in0=var[:], scalar1=1.0 / 128,
                                scalar2=1e-5, op0=ALU.mult, op1=ALU.add)
        std = pool.tile([4, 1], F32)
        nc.scalar.activation(out=std[:], in_=var[:], func=AF.Sqrt)
        rstd = pool.tile([4, 1], F32)
        nc.vector.reciprocal(out=rstd[:], in_=std[:])
        zn = pool.tile([4, 128], F32)
        nc.vector.tensor_scalar(out=zn[:], in0=zc[:], scalar1=rstd[:, 0:1],
                                scalar2=None, op0=ALU.mult)
        nc.vector.tensor_tensor(out=zn[:], in0=zn[:], in1=lng[:], op=ALU.mult)
        nc.vector.tensor_tensor(out=zn[:], in0=zn[:], in1=lnb[:], op=ALU.add)
        nc.scalar.activation(out=zn[:], in_=zn[:], func=AF.Relu)
        znT = pool.tile([128, 4], F32)
        tp2 = ppool.tile([128, 128], F32, tag="tp")
        nc.tensor.transpose(out=tp2[:, 0:4], in_=zn[:], identity=ident[0:4, 0:4])
        nc.vector.tensor_copy(out=znT[:], in_=tp2[:, 0:4])
        z2p = ppool.tile([4, 64], F32, tag="z2p")
        nc.tensor.matmul(out=z2p[:], lhsT=znT[:], rhs=p2a[:],
                         start=True, stop=False)
        nc.tensor.matmul(out=z2p[:], lhsT=onesg[:], rhs=p2c[:],
                         start=False, stop=True)
        zout = pool.tile([4, 64], F32)
        nc.scalar.activation(out=zout[:], in_=z2p[:], func=AF.Relu)
        nc.sync.dma_start(out=t_out[:], in_=zout[:])


# ----------------------------------------------------------------------------
# Entry point
# ----------------------------------------------------------------------------

def _pack_inputs(inp, cores, packs, nspans, rows_per_core, rows_total, x_aug_T):
    f16 = np.float16
    iota_row = np.broadcast_to(
        np.arange(128, dtype=f16)[None, :], (128, 128)
    ).copy()
    in_maps = []
    for k in range(NCORES):
        p = packs[k]
        m = {
            "xaugT": x_aug_T.astype(np.float32),
            "own_xaugT": np.ascontiguousarray(
                x_aug_T[:, p["own_cols"]]
            ).astype(np.float32),
            "enc_aug": np.concatenate(
                [np.asarray(inp["enc_w"], np.float32),
                 np.asarray(inp["enc_b"], np.float32)[None, :]], 0
            ),
            "idxsw": p["idxsw"],
            "idxqp": p["idxqp"],
            "swcnt": p["swcnt"],
            "metaF": p["metaF"],
            "iota_row": iota_row,
            "inv_cnt": p["inv_cnt"],
            "p1_aug": np.concatenate(
                [np.asarray(inp["p1_w"], np.float32),
                 np.asarray(inp["p1_b"], np.float32)[None, :]], 0
            ),
            "ln_g4": np.broadcast_to(
                np.asarray(inp["ln_g"], np.float32)[None, :], (4, 128)
            ).copy(),
            "ln_b4": np.broadcast_to(
                np.asarray(inp["ln_b"], np.float32)[None, :], (4, 128)
            ).copy(),
            "p2_aug": np.concatenate(
                [np.asarray(inp["p2_w"], np.float32),
                 np.asarray(inp["p2_b"], np.float32)[None, :]], 0
            ),
        }
        for L in (1, 2):
            wl = np.asarray(inp[f"g{L}_wl"], np.float32)
            bl = np.asarray(inp[f"g{L}_bl"], np.float32)
            wr = np.asarray(inp[f"g{L}_wr"], np.float32)
            br = np.asarray(inp[f"g{L}_br"], np.float32)
            bias = np.asarray(inp[f"g{L}_bias"], np.float32)
            m[f"wl{L}_aug"] = np.concatenate([wl, bl[None, :]], 0).astype(f16)
            m[f"wr{L}_aug"] = np.concatenate([wr, br[None, :]], 0).astype(f16)
            m[f"bias{L}_row"] = np.broadcast_to(
                bias.reshape(1, HC), (128, HC)
            ).astype(f16).copy()
            m[f"att{L}_row"] = np.broadcast_to(
                np.asarray(inp[f"g{L}_att"], np.float32).reshape(1, HC), (128, HC)
            ).astype(f16).copy()
            m[f"we{L}_row"] = np.broadcast_to(
                np.asarray(inp[f"g{L}_we"], np.float32).reshape(1, HC), (128, HC)
            ).astype(f16).copy()
        in_maps.append(m)
    return in_maps


def kernel(**inputs):
    cores, packs, nspans, rows_per_core, rows_total, x_aug_T, node_row = _host_prep(
        inputs
    )
    key = (nspans, rows_total)
    if key not in _PROGRAM_CACHE:
        _PROGRAM_CACHE[key] = _build_program(nspans, rows_total)
    nc = _PROGRAM_CACHE[key]
    in_maps = _pack_inputs(
        inputs, cores, packs, nspans, rows_per_core, rows_total, x_aug_T
    )
    res = run_bass_kernel_spmd(nc, in_maps, core_ids=list(range(NCORES)))
    out = np.concatenate([res.results[k]["out"] for k in range(NCORES)], axis=0)
    return out.astype(np.float32)


if __name__ == "__main__":
    data = dict(np.load("/root/problem/inputs_cache.npz"))
    out = kernel(**data)
    exp = np.load("/root/problem/expected_np.npy")
    rel = np.linalg.norm(out - exp) / np.linalg.norm(exp)
    print("rel err:", rel)
